# revision 36
# baseline (speedup 1.0000x reference)
"""FP8 block-quantized MoE MLP (16 experts, top-4 routing) on 8 Trainium2 cores.

Strategy (expert-parallel, host-prepped activations):
  Host: build routing tables from top_k_index; each core owns 2 experts.
    The dynamic per-token/per-128-group fp8-e4m3fn activation quant-dequant
    is computed EXACTLY on the host (ml_dtypes e4m3fn == jax CPU rounding),
    cast to fp16 and stored pre-transposed per 128-token tile, so the device
    does zero x-side transposes or quantization work. Weights are block-
    dequantized in f32 on the host and cast to fp16 in [K,O] layout.
  Device (per core, per 128-row tile):
    GEMM1 (8 K-blocks -> gate|up PSUM), silu(gate)*up, fp8 quant-dequant of
    the intermediate (hardware fp8e4 with the /2 exponent trick: TRN fp8e4
    max-normal 240 vs OCP 448, halving keeps RNE rounding identical), PE
    transpose, GEMM2, scale rows by routing weight, DMA out.
    Software pipeline: GEMM2 runs two tiles behind GEMM1 so the whole
    h-chain (ACT silu -> DVE quant -> PE transpose -> ACT copy) has a full
    GEMM1 of slack; PE never stalls on another engine.
  Host: scatter-add the weighted rows into the [2048, 1024] output.
"""

import numpy as np
import ml_dtypes

# Problem constants (hardcoded per the task contract).
T = 2048
H = 1024
I_DIM = 512
E = 16
TK = 4
G = 128
FP8_MAX = 448.0
NCORES = 8
E_LOC = E // NCORES          # experts per core
KB1 = H // 128               # 8 contraction blocks for gate_up
KB2 = I_DIM // 128           # 4 contraction blocks for down
O1 = 2 * I_DIM               # 1024
O2 = H                       # 1024

WARMUP = 44                  # PE warm transposes (clock-gate / p-state ramp)

_PROGRAM_CACHE: dict = {}


def _build_program(cap: int, do_compile: bool = True):
    import concourse.bass as bass
    import concourse.mybir as mybir
    from concourse import bacc
    from concourse.tile import TileContext
    from contextlib import ExitStack

    dt = mybir.dt
    F32 = dt.float32
    FP8 = dt.float8e4
    FP16 = dt.float16
    R = E_LOC * cap
    ntiles = R // 128
    tpe = cap // 128

    nc = bacc.Bacc("TRN2")
    xq_d = nc.dram_tensor("xq", [ntiles, 128, KB1, 128], FP16, kind="ExternalInput")
    rw_d = nc.dram_tensor("rw", [128, ntiles], F32, kind="ExternalInput")
    w1_d = nc.dram_tensor("w1", [E_LOC, 128, KB1, O1], FP16, kind="ExternalInput")
    w2_d = nc.dram_tensor("w2", [E_LOC, 128, KB2, O2], FP16, kind="ExternalInput")
    id_d = nc.dram_tensor("ident", [128, 128], FP16, kind="ExternalInput")
    out_d = nc.dram_tensor("out", [R, H], F32, kind="ExternalOutput")

    AX = mybir.AxisListType.X
    OP = mybir.AluOpType
    ACT = mybir.ActivationFunctionType

    def bcast(ap, reps):
        # [P, g] -> [P, g, reps] with a stride-0 innermost dim
        return bass.AP(tensor=ap.tensor, offset=ap.offset,
                       ap=[ap.ap[0], ap.ap[1], [0, reps]])

    def bcast_p1(ap, reps):
        # [P, 1] -> [P, reps] with a stride-0 free dim
        return bass.AP(tensor=ap.tensor, offset=ap.offset,
                       ap=[ap.ap[0], [0, reps]])

    with TileContext(nc) as tc, ExitStack() as ctx:
        singles = ctx.enter_context(tc.tile_pool(name="singles", bufs=1))
        xpool = ctx.enter_context(tc.tile_pool(name="xpool", bufs=4))
        spool = ctx.enter_context(tc.tile_pool(name="spool", bufs=8))
        qpool = ctx.enter_context(tc.tile_pool(name="qpool", bufs=3))
        dqpool = ctx.enter_context(tc.tile_pool(name="dqpool", bufs=3))
        tpool = ctx.enter_context(tc.tile_pool(name="tpool", bufs=3))
        hpool = ctx.enter_context(tc.tile_pool(name="hpool", bufs=3))
        opool = ctx.enter_context(tc.tile_pool(name="opool", bufs=3))
        ps_tx = ctx.enter_context(tc.tile_pool(name="ps_tx", bufs=2, space="PSUM"))
        ps_m1 = ctx.enter_context(tc.tile_pool(name="ps_m1", bufs=2, space="PSUM"))
        ps_m2 = ctx.enter_context(tc.tile_pool(name="ps_m2", bufs=1, space="PSUM"))

        state = {}
        ident = singles.tile([128, 128], FP16)
        w1_sb = singles.tile([128, E_LOC, KB1, O1], FP16)
        w2_sb = singles.tile([128, E_LOC, KB2, O2], FP16)
        rw_sb = singles.tile([128, ntiles], F32)

        # DMA issue plan (each dma_start blocks the issuing sequencer for
        # ~0.6 us, so keep the count low and off the busy engines; the ACT
        # queue is poisoned at its head by two implicit 1.3us
        # ACT_TABLE_LOADs, so nothing startup-critical goes there):
        #   SP : ident, w1[e0] kb 0-1, xq tile 0/1, kb 4-5, w2[e0], w1[e1],
        #        w2[e1], then per-tile out stores (SP is otherwise idle).
        #   DVE: rw, w1[e0] kb 2-3, kb 6-7 (issued before its first compute)
        #        so expert-0's gate_up weights arrive on two queues.
        #   GPS: remaining per-tile xq loads.
        def emit_load(it, q=None):
            x_t = xpool.tile([128, KB1, 128], FP16, tag="xq")
            (q or nc.gpsimd).dma_start(out=x_t, in_=xq_d[it])
            state[it] = {"xq": x_t}

        # Critical startup window: ONLY expert-0 gate_up weights + first xq
        # tiles. In-flight DMAs share bandwidth (not FIFO), so every byte
        # issued here delays GEMM1 of tile 0. Expert-1 weights are issued on
        # the gpsimd queue behind the xq tile-4 load, which cannot configure
        # until GEMM1 of tile 0 retires its buffer -- a natural delay.
        nc.sync.dma_start(out=ident, in_=id_d[:, :])
        nc.scalar.dma_start(out=rw_sb, in_=rw_d[:, :])
        nc.sync.dma_start(out=w1_sb[:, 0, 0], in_=w1_d[0, :, 0])
        nc.scalar.dma_start(out=w1_sb[:, 0, 1], in_=w1_d[0, :, 1])
        emit_load(0, q=nc.sync)
        nc.scalar.dma_start(out=w1_sb[:, 0, 3], in_=w1_d[0, :, 3])
        nc.sync.dma_start(out=w1_sb[:, 0, 2], in_=w1_d[0, :, 2])
        nc.scalar.dma_start(out=w1_sb[:, 0, 5], in_=w1_d[0, :, 5])
        nc.sync.dma_start(out=w1_sb[:, 0, 4], in_=w1_d[0, :, 4])
        if ntiles > 1:
            emit_load(1, q=nc.sync)
        nc.scalar.dma_start(out=w1_sb[:, 0, 7], in_=w1_d[0, :, 7])
        nc.sync.dma_start(out=w1_sb[:, 0, 6], in_=w1_d[0, :, 6])
        nc.sync.dma_start(out=w2_sb[:, 0], in_=w2_d[0])
        if E_LOC > 1:
            nc.sync.dma_start(out=w1_sb[:, 1], in_=w1_d[1])
            nc.sync.dma_start(out=w2_sb[:, 1], in_=w2_d[1])

        # PE warmup: open the HAM clock gate / ramp the p-state while the
        # first weight DMAs are in flight. Uses a memset scratch tile (not
        # the DMA'd identity) so it has no dependency on any input DMA.
        warm_in = singles.tile([128, 128], FP16)
        nc.gpsimd.memset(warm_in, 0.0)
        warm_ps = ps_tx.tile([128, KB2, 128], FP16, tag="ps_tx")
        for _ in range(WARMUP):
            nc.tensor.transpose(warm_ps[:, 0], warm_in, warm_in)

        def emit_g1(it):
            """GEMM1 for tile `it`: xqT (stationary, host-transposed) x w1."""
            e = it // tpe
            xq = state[it]["xq"]
            ps_gu = ps_m1.tile([128, 1024], F32, tag="ps_gu")
            # gate half first so silu can start at GEMM1's halfway point
            for kb in range(KB1):
                nc.tensor.matmul(ps_gu[:, 0:512], xq[:, kb], w1_sb[:, e, kb, 0:512],
                                 start=(kb == 0), stop=(kb == KB1 - 1))
            for kb in range(KB1):
                nc.tensor.matmul(ps_gu[:, 512:1024], xq[:, kb],
                                 w1_sb[:, e, kb, 512:1024],
                                 start=(kb == 0), stop=(kb == KB1 - 1))
            state[it]["gu"] = ps_gu

        def emit_silu(it):
            """ACT: silu(gate) -> SBUF (queued on ACT ahead of older copies)."""
            st = state[it]
            h_t = hpool.tile([128, I_DIM], F32, tag="h")
            nc.scalar.activation(out=h_t, in_=st["gu"][:, 0:512], func=ACT.Silu)
            st["h"] = h_t

        def emit_hchain(it):
            """DVE: h = silu*up, fp8 quant-dequant (the /2 trick), fp16 out."""
            st = state[it]
            h_t = st["h"]
            nc.vector.tensor_tensor(out=h_t, in0=h_t, in1=st["gu"][:, 512:1024],
                                    op=OP.mult)
            amax_h = spool.tile([128, KB2], F32, tag="amax_h")
            nc.vector.tensor_reduce(
                out=amax_h, in_=h_t.rearrange("p (g j) -> p g j", j=128),
                axis=AX, op=OP.max, apply_absolute_value=True)
            scl2h = spool.tile([128, KB2], F32, tag="scl_h")
            nc.vector.tensor_scalar(out=scl2h, in0=amax_h, scalar1=1e-10,
                                    scalar2=2.0 / FP8_MAX, op0=OP.max,
                                    op1=OP.mult)
            inv2h = spool.tile([128, KB2], F32, tag="inv_h")
            nc.vector.reciprocal(inv2h, scl2h)
            # fold the routing weight into the dequant scale
            # (rw * (hq @ w2) == (rw*hq) @ w2, modulo fp16 rounding of hq)
            sclrw = spool.tile([128, KB2], F32, tag="sclrw")
            nc.vector.tensor_tensor(out=sclrw, in0=scl2h,
                                    in1=bcast_p1(rw_sb[:, it:it + 1], KB2),
                                    op=OP.mult)
            # quantize + dequantize in kb-halves so the PE transposes of the
            # first half can start while DVE works on the second (tail path)
            q8h = qpool.tile([128, KB2, 128], FP8, tag="q8_h")
            hq = dqpool.tile([128, KB2, 128], FP16, tag="hq")
            hg = h_t.rearrange("p (g j) -> p g j", j=128)
            for ha in range(2):
                g = slice(ha * 2, ha * 2 + 2)
                nc.vector.tensor_tensor(out=q8h[:, g], in0=hg[:, g],
                                        in1=bcast(inv2h[:, g], 128), op=OP.mult)
                nc.vector.tensor_tensor(out=hq[:, g], in0=q8h[:, g],
                                        in1=bcast(sclrw[:, g], 128), op=OP.mult)
            st["hq"] = hq

        def emit_th(it):
            """PE transpose of hq + ACT copy to SBUF, in kb-halves."""
            st = state[it]
            psh = ps_tx.tile([128, KB2, 128], FP16, tag="ps_tx")
            hT = tpool.tile([128, KB2, 128], FP16, tag="hT")
            for ha in range(2):
                for kb in (ha * 2, ha * 2 + 1):
                    nc.tensor.transpose(psh[:, kb], st["hq"][:, kb, :], ident)
                g = slice(ha * 2, ha * 2 + 2)
                nc.scalar.copy(out=hT[:, g], in_=psh[:, g])
            st["hT"] = hT

        def emit_g2(it, tail=False, last=False):
            """GEMM2 (routing weight pre-folded into hq) + PSUM->DRAM store.

            tail=True (last two tiles): use the ps_m1 pool, which is free
            once GEMM1s are done -- ps_m2 (bufs=1) would stall on the
            previous store's drain with no GEMM1 left to cover it."""
            e = it // tpe
            r0 = it * 128
            st = state.pop(it)
            hT = st["hT"]
            pool = ps_m1 if tail else ps_m2
            ps_o = pool.tile([128, 1024], F32, tag="ps_gu" if tail else "ps_o")
            o_t = opool.tile([128, H], F32, tag="o")
            if tail:
                # very last tile: second half's store issues from the Scalar
                # queue so its DGE config overlaps Sync's first-half config
                for half, q in ((0, nc.sync), (1, nc.scalar if last else nc.sync)):
                    sl = slice(half * 512, half * 512 + 512)
                    for kb in range(KB2):
                        nc.tensor.matmul(ps_o[:, sl], hT[:, kb],
                                         w2_sb[:, e, kb, sl],
                                         start=(kb == 0), stop=(kb == KB2 - 1))
                    nc.scalar.activation(out=o_t[:, sl], in_=ps_o[:, sl],
                                         func=ACT.Copy)
                    q.dma_start(out=out_d[r0:r0 + 128, sl], in_=o_t[:, sl])
                return
            for kb in range(KB2):
                nc.tensor.matmul(ps_o[:, 0:512], hT[:, kb], w2_sb[:, e, kb, 0:512],
                                 start=(kb == 0), stop=(kb == KB2 - 1))
                nc.tensor.matmul(ps_o[:, 512:1024], hT[:, kb],
                                 w2_sb[:, e, kb, 512:1024],
                                 start=(kb == 0), stop=(kb == KB2 - 1))
            nc.scalar.activation(out=o_t[:, 0:512], in_=ps_o[:, 0:512],
                                 func=ACT.Copy)
            nc.scalar.activation(out=o_t[:, 512:1024], in_=ps_o[:, 512:1024],
                                 func=ACT.Copy)
            nc.sync.dma_start(out=out_d[r0:r0 + 128, :], in_=o_t)

        # Software pipeline (per-engine queues execute in emission order):
        #   PE  per step: G1_i, Th_{i-1}, G2_{i-2}
        #   ACT per step: silu_i, hTcopy_{i-1}, outcopy_{i-2}
        #   DVE per step: hchain_i
        # Every cross-engine dependency has >= one full GEMM1 of slack.
        if ntiles > 2:
            emit_load(2)
        for it in range(ntiles):
            emit_g1(it)
            emit_silu(it)
            if it >= 1:
                emit_th(it - 1)
            if it >= 2:
                emit_g2(it - 2)
            emit_hchain(it)
            if it + 3 < ntiles:
                emit_load(it + 3)
        # tail: G2(n-2) first (its hT is ready), then the transpose of the
        # last tile (which must wait for its h-chain), then a split G2.
        if ntiles >= 2:
            emit_g2(ntiles - 2, tail=True)
        emit_th(ntiles - 1)
        emit_g2(ntiles - 1, tail=True, last=True)

    if do_compile:
        nc.compile()
    return nc


def _get_program(cap: int):
    if cap not in _PROGRAM_CACHE:
        _PROGRAM_CACHE[cap] = _build_program(cap)
    return _PROGRAM_CACHE[cap]


def _dequant_weight(w, s, g=G):
    E_, O_, K_ = w.shape
    wb = w.reshape(E_, O_ // g, g, K_ // g, g)
    return (wb * s[:, :, None, :, None]).reshape(E_, O_, K_)


def _qdq_act(x):
    """Exact e4m3fn per-token/per-128-group quant-dequant (matches jax CPU)."""
    Tn, K = x.shape
    xg = x.reshape(Tn, K // G, G)
    amax = np.max(np.abs(xg), axis=-1, keepdims=True)
    scale = np.maximum(amax, 1e-10) / FP8_MAX
    q = np.clip(xg / scale, -FP8_MAX, FP8_MAX).astype(ml_dtypes.float8_e4m3fn)
    return (q.astype(np.float32) * scale).reshape(Tn, K)


def _prep(inputs):
    hs = np.ascontiguousarray(np.asarray(inputs["hidden_states"], np.float32))
    idx = np.asarray(inputs["top_k_index"]).astype(np.int64)
    tkw = np.asarray(inputs["top_k_weights"], np.float32)
    gup = np.asarray(inputs["gate_up_proj"], np.float32)
    gup_s = np.asarray(inputs["gate_up_proj_scale_inv"], np.float32)
    dn = np.asarray(inputs["down_proj"], np.float32)
    dn_s = np.asarray(inputs["down_proj_scale_inv"], np.float32)

    # routing tables: merge duplicate (token, expert) pairs (the reference
    # sums top-k weights per expert), then group by expert
    flat_e = idx.reshape(-1)
    flat_t = np.repeat(np.arange(T, dtype=np.int64), TK)
    flat_w = tkw.reshape(-1).astype(np.float64)
    key = flat_e * T + flat_t
    uk, inv = np.unique(key, return_inverse=True)
    sw = np.bincount(inv, weights=flat_w).astype(np.float32)
    se = (uk // T).astype(np.int64)
    st = (uk % T).astype(np.int64)
    counts = np.bincount(se, minlength=E)
    cap = int(np.ceil(max(int(counts.max()), 1) / 128.0) * 128)
    R = E_LOC * cap
    ntiles = R // 128

    starts = np.zeros(E + 1, np.int64)
    np.cumsum(counts, out=starts[1:])

    # exact activation quant-dequant once for all tokens, then fp16
    xq_full = _qdq_act(hs).astype(np.float16)          # [T, H]

    # weights: exact f32 block dequant, then fp16 in [K, O] layout
    w1_full = _dequant_weight(gup, gup_s)              # [E, O1, H]
    w1_t = np.ascontiguousarray(
        w1_full.transpose(0, 2, 1).reshape(E, KB1, 128, O1).transpose(0, 2, 1, 3)
    ).astype(np.float16)                               # [E, 128, KB1, O1]
    w2_full = _dequant_weight(dn, dn_s)                # [E, O2, I]
    w2_t = np.ascontiguousarray(
        w2_full.transpose(0, 2, 1).reshape(E, KB2, 128, O2).transpose(0, 2, 1, 3)
    ).astype(np.float16)                               # [E, 128, KB2, O2]

    ident = np.eye(128, dtype=np.float16)

    in_maps = []
    tok_core = []      # per-core valid token ids (concatenated per expert)
    nvalid_core = []   # per-core list of (row_offset, count)
    for c in range(NCORES):
        rows_idx = np.zeros(R, np.int64)
        rw_vec = np.zeros(R, np.float32)
        segs = []
        for j in range(E_LOC):
            e = c * E_LOC + j
            n = int(counts[e])
            s0, r0 = starts[e], j * cap
            rows_idx[r0:r0 + n] = st[s0:s0 + n]
            rw_vec[r0:r0 + n] = sw[s0:s0 + n]
            segs.append((r0, n))
        xg = xq_full[rows_idx]                         # [R, H] fp16
        # per-tile transpose: [ntiles, 128k, KB1, 128t]
        xqT = np.ascontiguousarray(
            xg.reshape(ntiles, 128, KB1, 128).transpose(0, 3, 2, 1))
        in_maps.append({
            "xq": xqT,
            "rw": np.ascontiguousarray(rw_vec.reshape(ntiles, 128).T),
            "w1": np.ascontiguousarray(w1_t[c * E_LOC:(c + 1) * E_LOC]),
            "w2": np.ascontiguousarray(w2_t[c * E_LOC:(c + 1) * E_LOC]),
            "ident": ident,
        })
        tok_core.append(rows_idx)
        nvalid_core.append(segs)
    return cap, in_maps, tok_core, nvalid_core


def _combine(results, tok_core, nvalid_core):
    out = np.zeros((T, H), np.float32)
    for c in range(NCORES):
        res = results[c]["out"]
        for (r0, n) in nvalid_core[c]:
            if n:
                np.add.at(out, tok_core[c][r0:r0 + n], res[r0:r0 + n])
    return out


def kernel_with_results(inputs, trace=False):
    from concourse.bass_utils import run_bass_kernel_spmd
    cap, in_maps, tok_core, nvalid_core = _prep(inputs)
    nc = _get_program(cap)
    bres = run_bass_kernel_spmd(nc, in_maps, core_ids=list(range(NCORES)),
                                trace=trace)
    out = _combine(bres.results, tok_core, nvalid_core)
    return out, bres


def kernel(**inputs) -> np.ndarray:
    out, _ = kernel_with_results(inputs, trace=False)
    return out


# revision 37
# speedup vs baseline: 1.0010x; 1.0010x over previous
"""FP8 block-quantized MoE MLP (16 experts, top-4 routing) on 8 Trainium2 cores.

Strategy (expert-parallel, host-prepped activations):
  Host: build routing tables from top_k_index; each core owns 2 experts.
    The dynamic per-token/per-128-group fp8-e4m3fn activation quant-dequant
    is computed EXACTLY on the host (ml_dtypes e4m3fn == jax CPU rounding),
    cast to fp16 and stored pre-transposed per 128-token tile, so the device
    does zero x-side transposes or quantization work. Weights are block-
    dequantized in f32 on the host and cast to fp16 in [K,O] layout.
  Device (per core, per 128-row tile):
    GEMM1 (8 K-blocks -> gate|up PSUM), silu(gate)*up, fp8 quant-dequant of
    the intermediate (hardware fp8e4 with the /2 exponent trick: TRN fp8e4
    max-normal 240 vs OCP 448, halving keeps RNE rounding identical), PE
    transpose, GEMM2, scale rows by routing weight, DMA out.
    Software pipeline: GEMM2 runs two tiles behind GEMM1 so the whole
    h-chain (ACT silu -> DVE quant -> PE transpose -> ACT copy) has a full
    GEMM1 of slack; PE never stalls on another engine.
  Host: scatter-add the weighted rows into the [2048, 1024] output.
"""

import numpy as np
import ml_dtypes

# Problem constants (hardcoded per the task contract).
T = 2048
H = 1024
I_DIM = 512
E = 16
TK = 4
G = 128
FP8_MAX = 448.0
NCORES = 8
E_LOC = E // NCORES          # experts per core
KB1 = H // 128               # 8 contraction blocks for gate_up
KB2 = I_DIM // 128           # 4 contraction blocks for down
O1 = 2 * I_DIM               # 1024
O2 = H                       # 1024

WARMUP = 36                  # PE warm transposes (clock-gate / p-state ramp)

_PROGRAM_CACHE: dict = {}


def _build_program(cap: int, do_compile: bool = True):
    import concourse.bass as bass
    import concourse.mybir as mybir
    from concourse import bacc
    from concourse.tile import TileContext
    from contextlib import ExitStack

    dt = mybir.dt
    F32 = dt.float32
    FP8 = dt.float8e4
    FP16 = dt.float16
    R = E_LOC * cap
    ntiles = R // 128
    tpe = cap // 128

    nc = bacc.Bacc("TRN2")
    xq_d = nc.dram_tensor("xq", [ntiles, 128, KB1, 128], FP16, kind="ExternalInput")
    rw_d = nc.dram_tensor("rw", [128, ntiles], F32, kind="ExternalInput")
    w1_d = nc.dram_tensor("w1", [E_LOC, 128, KB1, O1], FP16, kind="ExternalInput")
    w2_d = nc.dram_tensor("w2", [E_LOC, 128, KB2, O2], FP16, kind="ExternalInput")
    id_d = nc.dram_tensor("ident", [128, 128], FP16, kind="ExternalInput")
    out_d = nc.dram_tensor("out", [R, H], F32, kind="ExternalOutput")

    AX = mybir.AxisListType.X
    OP = mybir.AluOpType
    ACT = mybir.ActivationFunctionType

    def bcast(ap, reps):
        # [P, g] -> [P, g, reps] with a stride-0 innermost dim
        return bass.AP(tensor=ap.tensor, offset=ap.offset,
                       ap=[ap.ap[0], ap.ap[1], [0, reps]])

    def bcast_p1(ap, reps):
        # [P, 1] -> [P, reps] with a stride-0 free dim
        return bass.AP(tensor=ap.tensor, offset=ap.offset,
                       ap=[ap.ap[0], [0, reps]])

    with TileContext(nc) as tc, ExitStack() as ctx:
        singles = ctx.enter_context(tc.tile_pool(name="singles", bufs=1))
        xpool = ctx.enter_context(tc.tile_pool(name="xpool", bufs=4))
        spool = ctx.enter_context(tc.tile_pool(name="spool", bufs=8))
        qpool = ctx.enter_context(tc.tile_pool(name="qpool", bufs=3))
        dqpool = ctx.enter_context(tc.tile_pool(name="dqpool", bufs=3))
        tpool = ctx.enter_context(tc.tile_pool(name="tpool", bufs=3))
        hpool = ctx.enter_context(tc.tile_pool(name="hpool", bufs=3))
        opool = ctx.enter_context(tc.tile_pool(name="opool", bufs=3))
        ps_tx = ctx.enter_context(tc.tile_pool(name="ps_tx", bufs=2, space="PSUM"))
        ps_m1 = ctx.enter_context(tc.tile_pool(name="ps_m1", bufs=2, space="PSUM"))
        ps_m2 = ctx.enter_context(tc.tile_pool(name="ps_m2", bufs=1, space="PSUM"))

        state = {}
        ident = singles.tile([128, 128], FP16)
        w1_sb = singles.tile([128, E_LOC, KB1, O1], FP16)
        w2_sb = singles.tile([128, E_LOC, KB2, O2], FP16)
        rw_sb = singles.tile([128, ntiles], F32)

        # DMA issue plan (each dma_start blocks the issuing sequencer for
        # ~0.6 us, so keep the count low and off the busy engines; the ACT
        # queue is poisoned at its head by two implicit 1.3us
        # ACT_TABLE_LOADs, so nothing startup-critical goes there):
        #   SP : ident, w1[e0] kb 0-1, xq tile 0/1, kb 4-5, w2[e0], w1[e1],
        #        w2[e1], then per-tile out stores (SP is otherwise idle).
        #   DVE: rw, w1[e0] kb 2-3, kb 6-7 (issued before its first compute)
        #        so expert-0's gate_up weights arrive on two queues.
        #   GPS: remaining per-tile xq loads.
        def emit_load(it, q=None):
            x_t = xpool.tile([128, KB1, 128], FP16, tag="xq")
            (q or nc.gpsimd).dma_start(out=x_t, in_=xq_d[it])
            state[it] = {"xq": x_t}

        # Critical startup window: ONLY expert-0 gate_up weights + first xq
        # tiles. In-flight DMAs share bandwidth (not FIFO), so every byte
        # issued here delays GEMM1 of tile 0. Expert-1 weights are issued on
        # the gpsimd queue behind the xq tile-4 load, which cannot configure
        # until GEMM1 of tile 0 retires its buffer -- a natural delay.
        nc.sync.dma_start(out=ident, in_=id_d[:, :])
        nc.scalar.dma_start(out=rw_sb, in_=rw_d[:, :])
        nc.sync.dma_start(out=w1_sb[:, 0, 0], in_=w1_d[0, :, 0])
        nc.scalar.dma_start(out=w1_sb[:, 0, 1], in_=w1_d[0, :, 1])
        emit_load(0, q=nc.sync)
        nc.scalar.dma_start(out=w1_sb[:, 0, 3], in_=w1_d[0, :, 3])
        nc.sync.dma_start(out=w1_sb[:, 0, 2], in_=w1_d[0, :, 2])
        nc.scalar.dma_start(out=w1_sb[:, 0, 5], in_=w1_d[0, :, 5])
        nc.sync.dma_start(out=w1_sb[:, 0, 4], in_=w1_d[0, :, 4])
        if ntiles > 1:
            emit_load(1, q=nc.sync)
        nc.scalar.dma_start(out=w1_sb[:, 0, 7], in_=w1_d[0, :, 7])
        nc.sync.dma_start(out=w1_sb[:, 0, 6], in_=w1_d[0, :, 6])
        nc.sync.dma_start(out=w2_sb[:, 0], in_=w2_d[0])
        if E_LOC > 1:
            nc.sync.dma_start(out=w1_sb[:, 1], in_=w1_d[1])
            nc.sync.dma_start(out=w2_sb[:, 1], in_=w2_d[1])

        # PE warmup: open the HAM clock gate / ramp the p-state while the
        # first weight DMAs are in flight. Uses a memset scratch tile (not
        # the DMA'd identity) so it has no dependency on any input DMA.
        warm_in = singles.tile([128, 128], FP16)
        nc.gpsimd.memset(warm_in, 0.0)
        warm_ps = ps_tx.tile([128, KB2, 128], FP16, tag="ps_tx")
        for _ in range(WARMUP):
            nc.tensor.transpose(warm_ps[:, 0], warm_in, warm_in)

        def emit_g1(it):
            """GEMM1 for tile `it`: xqT (stationary, host-transposed) x w1."""
            e = it // tpe
            xq = state[it]["xq"]
            ps_gu = ps_m1.tile([128, 1024], F32, tag="ps_gu")
            # gate half first so silu can start at GEMM1's halfway point
            for kb in range(KB1):
                nc.tensor.matmul(ps_gu[:, 0:512], xq[:, kb], w1_sb[:, e, kb, 0:512],
                                 start=(kb == 0), stop=(kb == KB1 - 1))
            for kb in range(KB1):
                nc.tensor.matmul(ps_gu[:, 512:1024], xq[:, kb],
                                 w1_sb[:, e, kb, 512:1024],
                                 start=(kb == 0), stop=(kb == KB1 - 1))
            state[it]["gu"] = ps_gu

        def emit_silu(it):
            """ACT: silu(gate) -> SBUF (queued on ACT ahead of older copies)."""
            st = state[it]
            h_t = hpool.tile([128, I_DIM], F32, tag="h")
            nc.scalar.activation(out=h_t, in_=st["gu"][:, 0:512], func=ACT.Silu)
            st["h"] = h_t

        def emit_hchain(it):
            """DVE: h = silu*up, fp8 quant-dequant (the /2 trick), fp16 out."""
            st = state[it]
            h_t = st["h"]
            nc.vector.tensor_tensor(out=h_t, in0=h_t, in1=st["gu"][:, 512:1024],
                                    op=OP.mult)
            amax_h = spool.tile([128, KB2], F32, tag="amax_h")
            nc.vector.tensor_reduce(
                out=amax_h, in_=h_t.rearrange("p (g j) -> p g j", j=128),
                axis=AX, op=OP.max, apply_absolute_value=True)
            scl2h = spool.tile([128, KB2], F32, tag="scl_h")
            nc.vector.tensor_scalar(out=scl2h, in0=amax_h, scalar1=1e-10,
                                    scalar2=2.0 / FP8_MAX, op0=OP.max,
                                    op1=OP.mult)
            inv2h = spool.tile([128, KB2], F32, tag="inv_h")
            nc.vector.reciprocal(inv2h, scl2h)
            # fold the routing weight into the dequant scale
            # (rw * (hq @ w2) == (rw*hq) @ w2, modulo fp16 rounding of hq)
            sclrw = spool.tile([128, KB2], F32, tag="sclrw")
            nc.vector.tensor_tensor(out=sclrw, in0=scl2h,
                                    in1=bcast_p1(rw_sb[:, it:it + 1], KB2),
                                    op=OP.mult)
            # quantize + dequantize in kb-halves so the PE transposes of the
            # first half can start while DVE works on the second (tail path)
            q8h = qpool.tile([128, KB2, 128], FP8, tag="q8_h")
            hq = dqpool.tile([128, KB2, 128], FP16, tag="hq")
            hg = h_t.rearrange("p (g j) -> p g j", j=128)
            for ha in range(2):
                g = slice(ha * 2, ha * 2 + 2)
                nc.vector.tensor_tensor(out=q8h[:, g], in0=hg[:, g],
                                        in1=bcast(inv2h[:, g], 128), op=OP.mult)
                nc.vector.tensor_tensor(out=hq[:, g], in0=q8h[:, g],
                                        in1=bcast(sclrw[:, g], 128), op=OP.mult)
            st["hq"] = hq

        def emit_th(it):
            """PE transpose of hq + ACT copy to SBUF, in kb-halves."""
            st = state[it]
            psh = ps_tx.tile([128, KB2, 128], FP16, tag="ps_tx")
            hT = tpool.tile([128, KB2, 128], FP16, tag="hT")
            for ha in range(2):
                for kb in (ha * 2, ha * 2 + 1):
                    nc.tensor.transpose(psh[:, kb], st["hq"][:, kb, :], ident)
                g = slice(ha * 2, ha * 2 + 2)
                nc.scalar.copy(out=hT[:, g], in_=psh[:, g])
            st["hT"] = hT

        def emit_g2(it, tail=False, last=False):
            """GEMM2 (routing weight pre-folded into hq) + PSUM->DRAM store.

            tail=True (last two tiles): use the ps_m1 pool, which is free
            once GEMM1s are done -- ps_m2 (bufs=1) would stall on the
            previous store's drain with no GEMM1 left to cover it."""
            e = it // tpe
            r0 = it * 128
            st = state.pop(it)
            hT = st["hT"]
            pool = ps_m1 if tail else ps_m2
            ps_o = pool.tile([128, 1024], F32, tag="ps_gu" if tail else "ps_o")
            o_t = opool.tile([128, H], F32, tag="o")
            if tail:
                # very last tile: second half's store issues from the Scalar
                # queue so its DGE config overlaps Sync's first-half config
                for half, q in ((0, nc.sync), (1, nc.scalar if last else nc.sync)):
                    sl = slice(half * 512, half * 512 + 512)
                    for kb in range(KB2):
                        nc.tensor.matmul(ps_o[:, sl], hT[:, kb],
                                         w2_sb[:, e, kb, sl],
                                         start=(kb == 0), stop=(kb == KB2 - 1))
                    nc.scalar.activation(out=o_t[:, sl], in_=ps_o[:, sl],
                                         func=ACT.Copy)
                    q.dma_start(out=out_d[r0:r0 + 128, sl], in_=o_t[:, sl])
                return
            for kb in range(KB2):
                nc.tensor.matmul(ps_o[:, 0:512], hT[:, kb], w2_sb[:, e, kb, 0:512],
                                 start=(kb == 0), stop=(kb == KB2 - 1))
                nc.tensor.matmul(ps_o[:, 512:1024], hT[:, kb],
                                 w2_sb[:, e, kb, 512:1024],
                                 start=(kb == 0), stop=(kb == KB2 - 1))
            nc.scalar.activation(out=o_t[:, 0:512], in_=ps_o[:, 0:512],
                                 func=ACT.Copy)
            nc.scalar.activation(out=o_t[:, 512:1024], in_=ps_o[:, 512:1024],
                                 func=ACT.Copy)
            nc.sync.dma_start(out=out_d[r0:r0 + 128, :], in_=o_t)

        # Software pipeline (per-engine queues execute in emission order):
        #   PE  per step: G1_i, Th_{i-1}, G2_{i-2}
        #   ACT per step: silu_i, hTcopy_{i-1}, outcopy_{i-2}
        #   DVE per step: hchain_i
        # Every cross-engine dependency has >= one full GEMM1 of slack.
        if ntiles > 2:
            emit_load(2)
        for it in range(ntiles):
            emit_g1(it)
            emit_silu(it)
            if it >= 1:
                emit_th(it - 1)
            if it >= 2:
                emit_g2(it - 2)
            emit_hchain(it)
            if it + 3 < ntiles:
                emit_load(it + 3)
        # tail: G2(n-2) first (its hT is ready), then the transpose of the
        # last tile (which must wait for its h-chain), then a split G2.
        if ntiles >= 2:
            emit_g2(ntiles - 2, tail=True)
        emit_th(ntiles - 1)
        emit_g2(ntiles - 1, tail=True, last=True)

    if do_compile:
        nc.compile()
    return nc


def _get_program(cap: int):
    if cap not in _PROGRAM_CACHE:
        _PROGRAM_CACHE[cap] = _build_program(cap)
    return _PROGRAM_CACHE[cap]


def _dequant_weight(w, s, g=G):
    E_, O_, K_ = w.shape
    wb = w.reshape(E_, O_ // g, g, K_ // g, g)
    return (wb * s[:, :, None, :, None]).reshape(E_, O_, K_)


def _qdq_act(x):
    """Exact e4m3fn per-token/per-128-group quant-dequant (matches jax CPU)."""
    Tn, K = x.shape
    xg = x.reshape(Tn, K // G, G)
    amax = np.max(np.abs(xg), axis=-1, keepdims=True)
    scale = np.maximum(amax, 1e-10) / FP8_MAX
    q = np.clip(xg / scale, -FP8_MAX, FP8_MAX).astype(ml_dtypes.float8_e4m3fn)
    return (q.astype(np.float32) * scale).reshape(Tn, K)


def _prep(inputs):
    hs = np.ascontiguousarray(np.asarray(inputs["hidden_states"], np.float32))
    idx = np.asarray(inputs["top_k_index"]).astype(np.int64)
    tkw = np.asarray(inputs["top_k_weights"], np.float32)
    gup = np.asarray(inputs["gate_up_proj"], np.float32)
    gup_s = np.asarray(inputs["gate_up_proj_scale_inv"], np.float32)
    dn = np.asarray(inputs["down_proj"], np.float32)
    dn_s = np.asarray(inputs["down_proj_scale_inv"], np.float32)

    # routing tables: merge duplicate (token, expert) pairs (the reference
    # sums top-k weights per expert), then group by expert
    flat_e = idx.reshape(-1)
    flat_t = np.repeat(np.arange(T, dtype=np.int64), TK)
    flat_w = tkw.reshape(-1).astype(np.float64)
    key = flat_e * T + flat_t
    uk, inv = np.unique(key, return_inverse=True)
    sw = np.bincount(inv, weights=flat_w).astype(np.float32)
    se = (uk // T).astype(np.int64)
    st = (uk % T).astype(np.int64)
    counts = np.bincount(se, minlength=E)
    cap = int(np.ceil(max(int(counts.max()), 1) / 128.0) * 128)
    R = E_LOC * cap
    ntiles = R // 128

    starts = np.zeros(E + 1, np.int64)
    np.cumsum(counts, out=starts[1:])

    # exact activation quant-dequant once for all tokens, then fp16
    xq_full = _qdq_act(hs).astype(np.float16)          # [T, H]

    # weights: exact f32 block dequant, then fp16 in [K, O] layout
    w1_full = _dequant_weight(gup, gup_s)              # [E, O1, H]
    w1_t = np.ascontiguousarray(
        w1_full.transpose(0, 2, 1).reshape(E, KB1, 128, O1).transpose(0, 2, 1, 3)
    ).astype(np.float16)                               # [E, 128, KB1, O1]
    w2_full = _dequant_weight(dn, dn_s)                # [E, O2, I]
    w2_t = np.ascontiguousarray(
        w2_full.transpose(0, 2, 1).reshape(E, KB2, 128, O2).transpose(0, 2, 1, 3)
    ).astype(np.float16)                               # [E, 128, KB2, O2]

    ident = np.eye(128, dtype=np.float16)

    in_maps = []
    tok_core = []      # per-core valid token ids (concatenated per expert)
    nvalid_core = []   # per-core list of (row_offset, count)
    for c in range(NCORES):
        rows_idx = np.zeros(R, np.int64)
        rw_vec = np.zeros(R, np.float32)
        segs = []
        for j in range(E_LOC):
            e = c * E_LOC + j
            n = int(counts[e])
            s0, r0 = starts[e], j * cap
            rows_idx[r0:r0 + n] = st[s0:s0 + n]
            rw_vec[r0:r0 + n] = sw[s0:s0 + n]
            segs.append((r0, n))
        xg = xq_full[rows_idx]                         # [R, H] fp16
        # per-tile transpose: [ntiles, 128k, KB1, 128t]
        xqT = np.ascontiguousarray(
            xg.reshape(ntiles, 128, KB1, 128).transpose(0, 3, 2, 1))
        in_maps.append({
            "xq": xqT,
            "rw": np.ascontiguousarray(rw_vec.reshape(ntiles, 128).T),
            "w1": np.ascontiguousarray(w1_t[c * E_LOC:(c + 1) * E_LOC]),
            "w2": np.ascontiguousarray(w2_t[c * E_LOC:(c + 1) * E_LOC]),
            "ident": ident,
        })
        tok_core.append(rows_idx)
        nvalid_core.append(segs)
    return cap, in_maps, tok_core, nvalid_core


def _combine(results, tok_core, nvalid_core):
    out = np.zeros((T, H), np.float32)
    for c in range(NCORES):
        res = results[c]["out"]
        for (r0, n) in nvalid_core[c]:
            if n:
                np.add.at(out, tok_core[c][r0:r0 + n], res[r0:r0 + n])
    return out


def kernel_with_results(inputs, trace=False):
    from concourse.bass_utils import run_bass_kernel_spmd
    cap, in_maps, tok_core, nvalid_core = _prep(inputs)
    nc = _get_program(cap)
    bres = run_bass_kernel_spmd(nc, in_maps, core_ids=list(range(NCORES)),
                                trace=trace)
    out = _combine(bres.results, tok_core, nvalid_core)
    return out, bres


def kernel(**inputs) -> np.ndarray:
    out, _ = kernel_with_results(inputs, trace=False)
    return out


# revision 38
# speedup vs baseline: 1.0162x; 1.0152x over previous
"""FP8 block-quantized MoE MLP (16 experts, top-4 routing) on 8 Trainium2 cores.

Strategy (expert-parallel, host-prepped activations):
  Host: build routing tables from top_k_index; each core owns 2 experts.
    The dynamic per-token/per-128-group fp8-e4m3fn activation quant-dequant
    is computed EXACTLY on the host (ml_dtypes e4m3fn == jax CPU rounding),
    cast to fp16 and stored pre-transposed per 128-token tile, so the device
    does zero x-side transposes or quantization work. Weights are block-
    dequantized in f32 on the host and cast to fp16 in [K,O] layout.
  Device (per core, per 128-row tile):
    GEMM1 (8 K-blocks -> gate|up PSUM), silu(gate)*up, fp8 quant-dequant of
    the intermediate (hardware fp8e4 with the /2 exponent trick: TRN fp8e4
    max-normal 240 vs OCP 448, halving keeps RNE rounding identical), PE
    transpose, GEMM2, scale rows by routing weight, DMA out.
    Software pipeline: GEMM2 runs two tiles behind GEMM1 so the whole
    h-chain (ACT silu -> DVE quant -> PE transpose -> ACT copy) has a full
    GEMM1 of slack; PE never stalls on another engine.
  Host: scatter-add the weighted rows into the [2048, 1024] output.
"""

import numpy as np
import ml_dtypes

# Problem constants (hardcoded per the task contract).
T = 2048
H = 1024
I_DIM = 512
E = 16
TK = 4
G = 128
FP8_MAX = 448.0
NCORES = 8
E_LOC = E // NCORES          # experts per core
KB1 = H // 128               # 8 contraction blocks for gate_up
KB2 = I_DIM // 128           # 4 contraction blocks for down
O1 = 2 * I_DIM               # 1024
O2 = H                       # 1024

WARMUP = 32                  # PE warm transposes (clock-gate / p-state ramp)

_PROGRAM_CACHE: dict = {}


def _build_program(cap: int, do_compile: bool = True):
    import concourse.bass as bass
    import concourse.mybir as mybir
    from concourse import bacc
    from concourse.tile import TileContext
    from contextlib import ExitStack

    dt = mybir.dt
    F32 = dt.float32
    FP8 = dt.float8e4
    FP16 = dt.float16
    R = E_LOC * cap
    ntiles = R // 128
    tpe = cap // 128

    nc = bacc.Bacc("TRN2")
    xq_d = nc.dram_tensor("xq", [ntiles, 128, KB1, 128], FP16, kind="ExternalInput")
    rw_d = nc.dram_tensor("rw", [128, ntiles], F32, kind="ExternalInput")
    w1_d = nc.dram_tensor("w1", [E_LOC, 128, KB1, O1], FP16, kind="ExternalInput")
    w2_d = nc.dram_tensor("w2", [E_LOC, 128, KB2, O2], FP16, kind="ExternalInput")
    id_d = nc.dram_tensor("ident", [128, 128], FP16, kind="ExternalInput")
    out_d = nc.dram_tensor("out", [R, H], F32, kind="ExternalOutput")

    AX = mybir.AxisListType.X
    OP = mybir.AluOpType
    ACT = mybir.ActivationFunctionType

    def bcast(ap, reps):
        # [P, g] -> [P, g, reps] with a stride-0 innermost dim
        return bass.AP(tensor=ap.tensor, offset=ap.offset,
                       ap=[ap.ap[0], ap.ap[1], [0, reps]])

    def bcast_p1(ap, reps):
        # [P, 1] -> [P, reps] with a stride-0 free dim
        return bass.AP(tensor=ap.tensor, offset=ap.offset,
                       ap=[ap.ap[0], [0, reps]])

    with TileContext(nc) as tc, ExitStack() as ctx:
        singles = ctx.enter_context(tc.tile_pool(name="singles", bufs=1))
        xpool = ctx.enter_context(tc.tile_pool(name="xpool", bufs=4))
        spool = ctx.enter_context(tc.tile_pool(name="spool", bufs=8))
        qpool = ctx.enter_context(tc.tile_pool(name="qpool", bufs=3))
        dqpool = ctx.enter_context(tc.tile_pool(name="dqpool", bufs=3))
        tpool = ctx.enter_context(tc.tile_pool(name="tpool", bufs=3))
        hpool = ctx.enter_context(tc.tile_pool(name="hpool", bufs=3))
        opool = ctx.enter_context(tc.tile_pool(name="opool", bufs=3))
        ps_tx = ctx.enter_context(tc.tile_pool(name="ps_tx", bufs=2, space="PSUM"))
        ps_m1 = ctx.enter_context(tc.tile_pool(name="ps_m1", bufs=2, space="PSUM"))
        ps_m2 = ctx.enter_context(tc.tile_pool(name="ps_m2", bufs=1, space="PSUM"))

        state = {}
        ident = singles.tile([128, 128], FP16)
        w1_sb = singles.tile([128, E_LOC, KB1, O1], FP16)
        w2_sb = singles.tile([128, E_LOC, KB2, O2], FP16)
        rw_sb = singles.tile([128, ntiles], F32)

        # DMA issue plan (each dma_start blocks the issuing sequencer for
        # ~0.6 us, so keep the count low and off the busy engines; the ACT
        # queue is poisoned at its head by two implicit 1.3us
        # ACT_TABLE_LOADs, so nothing startup-critical goes there):
        #   SP : ident, w1[e0] kb 0-1, xq tile 0/1, kb 4-5, w2[e0], w1[e1],
        #        w2[e1], then per-tile out stores (SP is otherwise idle).
        #   DVE: rw, w1[e0] kb 2-3, kb 6-7 (issued before its first compute)
        #        so expert-0's gate_up weights arrive on two queues.
        #   GPS: remaining per-tile xq loads.
        def emit_load(it, q=None):
            x_t = xpool.tile([128, KB1, 128], FP16, tag="xq")
            (q or nc.gpsimd).dma_start(out=x_t, in_=xq_d[it])
            state[it] = {"xq": x_t}

        # Critical startup window: ONLY expert-0 gate_up weights + first xq
        # tiles. In-flight DMAs share bandwidth (not FIFO), so every byte
        # issued here delays GEMM1 of tile 0. Expert-1 weights are issued on
        # the gpsimd queue behind the xq tile-4 load, which cannot configure
        # until GEMM1 of tile 0 retires its buffer -- a natural delay.
        nc.sync.dma_start(out=ident, in_=id_d[:, :])
        nc.scalar.dma_start(out=rw_sb, in_=rw_d[:, :])
        nc.sync.dma_start(out=w1_sb[:, 0, 0], in_=w1_d[0, :, 0])
        nc.scalar.dma_start(out=w1_sb[:, 0, 1], in_=w1_d[0, :, 1])
        emit_load(0, q=nc.sync)
        nc.scalar.dma_start(out=w1_sb[:, 0, 3], in_=w1_d[0, :, 3])
        nc.sync.dma_start(out=w1_sb[:, 0, 2], in_=w1_d[0, :, 2])
        nc.scalar.dma_start(out=w1_sb[:, 0, 5], in_=w1_d[0, :, 5])
        nc.sync.dma_start(out=w1_sb[:, 0, 4], in_=w1_d[0, :, 4])
        if ntiles > 1:
            emit_load(1, q=nc.sync)
        nc.scalar.dma_start(out=w1_sb[:, 0, 7], in_=w1_d[0, :, 7])
        nc.sync.dma_start(out=w1_sb[:, 0, 6], in_=w1_d[0, :, 6])
        nc.sync.dma_start(out=w2_sb[:, 0], in_=w2_d[0])
        if E_LOC > 1:
            nc.sync.dma_start(out=w1_sb[:, 1], in_=w1_d[1])
            nc.sync.dma_start(out=w2_sb[:, 1], in_=w2_d[1])

        # PE warmup: open the HAM clock gate / ramp the p-state while the
        # first weight DMAs are in flight. Uses a memset scratch tile (not
        # the DMA'd identity) so it has no dependency on any input DMA.
        warm_in = singles.tile([128, 128], FP16)
        nc.gpsimd.memset(warm_in, 0.0)
        warm_ps = ps_tx.tile([128, KB2, 128], FP16, tag="ps_tx")
        for _ in range(WARMUP):
            nc.tensor.transpose(warm_ps[:, 0], warm_in, warm_in)

        def emit_g1(it):
            """GEMM1 for tile `it`: xqT (stationary, host-transposed) x w1."""
            e = it // tpe
            xq = state[it]["xq"]
            ps_gu = ps_m1.tile([128, 1024], F32, tag="ps_gu")
            # gate half first so silu can start at GEMM1's halfway point
            for kb in range(KB1):
                nc.tensor.matmul(ps_gu[:, 0:512], xq[:, kb], w1_sb[:, e, kb, 0:512],
                                 start=(kb == 0), stop=(kb == KB1 - 1))
            for kb in range(KB1):
                nc.tensor.matmul(ps_gu[:, 512:1024], xq[:, kb],
                                 w1_sb[:, e, kb, 512:1024],
                                 start=(kb == 0), stop=(kb == KB1 - 1))
            state[it]["gu"] = ps_gu

        def emit_silu(it):
            """ACT: silu(gate) -> SBUF (queued on ACT ahead of older copies)."""
            st = state[it]
            h_t = hpool.tile([128, I_DIM], F32, tag="h")
            nc.scalar.activation(out=h_t, in_=st["gu"][:, 0:512], func=ACT.Silu)
            st["h"] = h_t

        def emit_hchain(it):
            """DVE: h = silu*up, fp8 quant-dequant (the /2 trick), fp16 out."""
            st = state[it]
            h_t = st["h"]
            nc.vector.tensor_tensor(out=h_t, in0=h_t, in1=st["gu"][:, 512:1024],
                                    op=OP.mult)
            amax_h = spool.tile([128, KB2], F32, tag="amax_h")
            nc.vector.tensor_reduce(
                out=amax_h, in_=h_t.rearrange("p (g j) -> p g j", j=128),
                axis=AX, op=OP.max, apply_absolute_value=True)
            scl2h = spool.tile([128, KB2], F32, tag="scl_h")
            nc.vector.tensor_scalar(out=scl2h, in0=amax_h, scalar1=1e-10,
                                    scalar2=2.0 / FP8_MAX, op0=OP.max,
                                    op1=OP.mult)
            inv2h = spool.tile([128, KB2], F32, tag="inv_h")
            nc.vector.reciprocal(inv2h, scl2h)
            # fold the routing weight into the dequant scale
            # (rw * (hq @ w2) == (rw*hq) @ w2, modulo fp16 rounding of hq)
            sclrw = spool.tile([128, KB2], F32, tag="sclrw")
            nc.vector.tensor_tensor(out=sclrw, in0=scl2h,
                                    in1=bcast_p1(rw_sb[:, it:it + 1], KB2),
                                    op=OP.mult)
            # quantize + dequantize in kb-halves so the PE transposes of the
            # first half can start while DVE works on the second (tail path)
            q8h = qpool.tile([128, KB2, 128], FP8, tag="q8_h")
            hq = dqpool.tile([128, KB2, 128], FP16, tag="hq")
            hg = h_t.rearrange("p (g j) -> p g j", j=128)
            for ha in range(2):
                g = slice(ha * 2, ha * 2 + 2)
                nc.vector.tensor_tensor(out=q8h[:, g], in0=hg[:, g],
                                        in1=bcast(inv2h[:, g], 128), op=OP.mult)
                nc.vector.tensor_tensor(out=hq[:, g], in0=q8h[:, g],
                                        in1=bcast(sclrw[:, g], 128), op=OP.mult)
            st["hq"] = hq

        def emit_th(it):
            """PE transpose of hq + ACT copy to SBUF, in kb-halves."""
            st = state[it]
            psh = ps_tx.tile([128, KB2, 128], FP16, tag="ps_tx")
            hT = tpool.tile([128, KB2, 128], FP16, tag="hT")
            for ha in range(2):
                for kb in (ha * 2, ha * 2 + 1):
                    nc.tensor.transpose(psh[:, kb], st["hq"][:, kb, :], ident)
                g = slice(ha * 2, ha * 2 + 2)
                nc.scalar.copy(out=hT[:, g], in_=psh[:, g])
            st["hT"] = hT

        def emit_g2(it, tail=False):
            """GEMM2 (routing weight pre-folded into hq) + PSUM->DRAM store.

            tail=True (last two tiles): use the ps_m1 pool, which is free
            once GEMM1s are done -- ps_m2 (bufs=1) would stall on the
            previous store's drain with no GEMM1 left to cover it."""
            e = it // tpe
            r0 = it * 128
            st = state.pop(it)
            hT = st["hT"]
            pool = ps_m1 if tail else ps_m2
            ps_o = pool.tile([128, 1024], F32, tag="ps_gu" if tail else "ps_o")
            o_t = opool.tile([128, H], F32, tag="o")
            if tail:
                for half in range(2):
                    sl = slice(half * 512, half * 512 + 512)
                    for kb in range(KB2):
                        nc.tensor.matmul(ps_o[:, sl], hT[:, kb],
                                         w2_sb[:, e, kb, sl],
                                         start=(kb == 0), stop=(kb == KB2 - 1))
                    nc.scalar.activation(out=o_t[:, sl], in_=ps_o[:, sl],
                                         func=ACT.Copy)
                    nc.sync.dma_start(out=out_d[r0:r0 + 128, sl], in_=o_t[:, sl])
                return
            for kb in range(KB2):
                nc.tensor.matmul(ps_o[:, 0:512], hT[:, kb], w2_sb[:, e, kb, 0:512],
                                 start=(kb == 0), stop=(kb == KB2 - 1))
                nc.tensor.matmul(ps_o[:, 512:1024], hT[:, kb],
                                 w2_sb[:, e, kb, 512:1024],
                                 start=(kb == 0), stop=(kb == KB2 - 1))
            nc.scalar.activation(out=o_t[:, 0:512], in_=ps_o[:, 0:512],
                                 func=ACT.Copy)
            nc.scalar.activation(out=o_t[:, 512:1024], in_=ps_o[:, 512:1024],
                                 func=ACT.Copy)
            nc.sync.dma_start(out=out_d[r0:r0 + 128, :], in_=o_t)

        # Software pipeline (per-engine queues execute in emission order):
        #   PE  per step: G1_i, Th_{i-1}, G2_{i-2}
        #   ACT per step: silu_i, hTcopy_{i-1}, outcopy_{i-2}
        #   DVE per step: hchain_i
        # Every cross-engine dependency has >= one full GEMM1 of slack.
        if ntiles > 2:
            emit_load(2)
        for it in range(ntiles):
            emit_g1(it)
            emit_silu(it)
            if it >= 1:
                emit_th(it - 1)
            if it >= 2:
                emit_g2(it - 2)
            emit_hchain(it)
            if it + 3 < ntiles:
                emit_load(it + 3)
        # tail: G2(n-2) first (its hT is ready), then the transpose of the
        # last tile (which must wait for its h-chain), then a split G2.
        if ntiles >= 2:
            emit_g2(ntiles - 2, tail=True)
        emit_th(ntiles - 1)
        emit_g2(ntiles - 1, tail=True)

    if do_compile:
        nc.compile()
    return nc


def _get_program(cap: int):
    if cap not in _PROGRAM_CACHE:
        _PROGRAM_CACHE[cap] = _build_program(cap)
    return _PROGRAM_CACHE[cap]


def _dequant_weight(w, s, g=G):
    E_, O_, K_ = w.shape
    wb = w.reshape(E_, O_ // g, g, K_ // g, g)
    return (wb * s[:, :, None, :, None]).reshape(E_, O_, K_)


def _qdq_act(x):
    """Exact e4m3fn per-token/per-128-group quant-dequant (matches jax CPU)."""
    Tn, K = x.shape
    xg = x.reshape(Tn, K // G, G)
    amax = np.max(np.abs(xg), axis=-1, keepdims=True)
    scale = np.maximum(amax, 1e-10) / FP8_MAX
    q = np.clip(xg / scale, -FP8_MAX, FP8_MAX).astype(ml_dtypes.float8_e4m3fn)
    return (q.astype(np.float32) * scale).reshape(Tn, K)


def _prep(inputs):
    hs = np.ascontiguousarray(np.asarray(inputs["hidden_states"], np.float32))
    idx = np.asarray(inputs["top_k_index"]).astype(np.int64)
    tkw = np.asarray(inputs["top_k_weights"], np.float32)
    gup = np.asarray(inputs["gate_up_proj"], np.float32)
    gup_s = np.asarray(inputs["gate_up_proj_scale_inv"], np.float32)
    dn = np.asarray(inputs["down_proj"], np.float32)
    dn_s = np.asarray(inputs["down_proj_scale_inv"], np.float32)

    # routing tables: merge duplicate (token, expert) pairs (the reference
    # sums top-k weights per expert), then group by expert
    flat_e = idx.reshape(-1)
    flat_t = np.repeat(np.arange(T, dtype=np.int64), TK)
    flat_w = tkw.reshape(-1).astype(np.float64)
    key = flat_e * T + flat_t
    uk, inv = np.unique(key, return_inverse=True)
    sw = np.bincount(inv, weights=flat_w).astype(np.float32)
    se = (uk // T).astype(np.int64)
    st = (uk % T).astype(np.int64)
    counts = np.bincount(se, minlength=E)
    cap = int(np.ceil(max(int(counts.max()), 1) / 128.0) * 128)
    R = E_LOC * cap
    ntiles = R // 128

    starts = np.zeros(E + 1, np.int64)
    np.cumsum(counts, out=starts[1:])

    # exact activation quant-dequant once for all tokens, then fp16
    xq_full = _qdq_act(hs).astype(np.float16)          # [T, H]

    # weights: exact f32 block dequant, then fp16 in [K, O] layout
    w1_full = _dequant_weight(gup, gup_s)              # [E, O1, H]
    w1_t = np.ascontiguousarray(
        w1_full.transpose(0, 2, 1).reshape(E, KB1, 128, O1).transpose(0, 2, 1, 3)
    ).astype(np.float16)                               # [E, 128, KB1, O1]
    w2_full = _dequant_weight(dn, dn_s)                # [E, O2, I]
    w2_t = np.ascontiguousarray(
        w2_full.transpose(0, 2, 1).reshape(E, KB2, 128, O2).transpose(0, 2, 1, 3)
    ).astype(np.float16)                               # [E, 128, KB2, O2]

    ident = np.eye(128, dtype=np.float16)

    in_maps = []
    tok_core = []      # per-core valid token ids (concatenated per expert)
    nvalid_core = []   # per-core list of (row_offset, count)
    for c in range(NCORES):
        rows_idx = np.zeros(R, np.int64)
        rw_vec = np.zeros(R, np.float32)
        segs = []
        for j in range(E_LOC):
            e = c * E_LOC + j
            n = int(counts[e])
            s0, r0 = starts[e], j * cap
            rows_idx[r0:r0 + n] = st[s0:s0 + n]
            rw_vec[r0:r0 + n] = sw[s0:s0 + n]
            segs.append((r0, n))
        xg = xq_full[rows_idx]                         # [R, H] fp16
        # per-tile transpose: [ntiles, 128k, KB1, 128t]
        xqT = np.ascontiguousarray(
            xg.reshape(ntiles, 128, KB1, 128).transpose(0, 3, 2, 1))
        in_maps.append({
            "xq": xqT,
            "rw": np.ascontiguousarray(rw_vec.reshape(ntiles, 128).T),
            "w1": np.ascontiguousarray(w1_t[c * E_LOC:(c + 1) * E_LOC]),
            "w2": np.ascontiguousarray(w2_t[c * E_LOC:(c + 1) * E_LOC]),
            "ident": ident,
        })
        tok_core.append(rows_idx)
        nvalid_core.append(segs)
    return cap, in_maps, tok_core, nvalid_core


def _combine(results, tok_core, nvalid_core):
    out = np.zeros((T, H), np.float32)
    for c in range(NCORES):
        res = results[c]["out"]
        for (r0, n) in nvalid_core[c]:
            if n:
                np.add.at(out, tok_core[c][r0:r0 + n], res[r0:r0 + n])
    return out


def kernel_with_results(inputs, trace=False):
    from concourse.bass_utils import run_bass_kernel_spmd
    cap, in_maps, tok_core, nvalid_core = _prep(inputs)
    nc = _get_program(cap)
    bres = run_bass_kernel_spmd(nc, in_maps, core_ids=list(range(NCORES)),
                                trace=trace)
    out = _combine(bres.results, tok_core, nvalid_core)
    return out, bres


def kernel(**inputs) -> np.ndarray:
    out, _ = kernel_with_results(inputs, trace=False)
    return out


# revision 40
# speedup vs baseline: 1.0188x; 1.0026x over previous
"""FP8 block-quantized MoE MLP (16 experts, top-4 routing) on 8 Trainium2 cores.

Strategy (expert-parallel, host-prepped activations):
  Host: build routing tables from top_k_index; each core owns 2 experts.
    The dynamic per-token/per-128-group fp8-e4m3fn activation quant-dequant
    is computed EXACTLY on the host (ml_dtypes e4m3fn == jax CPU rounding),
    cast to fp16 and stored pre-transposed per 128-token tile, so the device
    does zero x-side transposes or quantization work. Weights are block-
    dequantized in f32 on the host and cast to fp16 in [K,O] layout.
  Device (per core, per 128-row tile):
    GEMM1 (8 K-blocks -> gate|up PSUM), silu(gate)*up, fp8 quant-dequant of
    the intermediate (hardware fp8e4 with the /2 exponent trick: TRN fp8e4
    max-normal 240 vs OCP 448, halving keeps RNE rounding identical), PE
    transpose, GEMM2, scale rows by routing weight, DMA out.
    Software pipeline: GEMM2 runs two tiles behind GEMM1 so the whole
    h-chain (ACT silu -> DVE quant -> PE transpose -> ACT copy) has a full
    GEMM1 of slack; PE never stalls on another engine.
  Host: scatter-add the weighted rows into the [2048, 1024] output.
"""

import numpy as np
import ml_dtypes

# Problem constants (hardcoded per the task contract).
T = 2048
H = 1024
I_DIM = 512
E = 16
TK = 4
G = 128
FP8_MAX = 448.0
NCORES = 8
E_LOC = E // NCORES          # experts per core
KB1 = H // 128               # 8 contraction blocks for gate_up
KB2 = I_DIM // 128           # 4 contraction blocks for down
O1 = 2 * I_DIM               # 1024
O2 = H                       # 1024

WARMUP = 32                  # PE warm transposes (clock-gate / p-state ramp)

_PROGRAM_CACHE: dict = {}


def _build_program(cap: int, do_compile: bool = True):
    import concourse.bass as bass
    import concourse.mybir as mybir
    from concourse import bacc
    from concourse.tile import TileContext
    from contextlib import ExitStack

    dt = mybir.dt
    F32 = dt.float32
    FP8 = dt.float8e4
    FP16 = dt.float16
    R = E_LOC * cap
    ntiles = R // 128
    tpe = cap // 128

    nc = bacc.Bacc("TRN2")
    xq_d = nc.dram_tensor("xq", [ntiles, 128, KB1, 128], FP16, kind="ExternalInput")
    rw_d = nc.dram_tensor("rw", [128, ntiles], F32, kind="ExternalInput")
    w1_d = nc.dram_tensor("w1", [E_LOC, 128, KB1, O1], FP16, kind="ExternalInput")
    w2_d = nc.dram_tensor("w2", [E_LOC, 128, KB2, O2], FP16, kind="ExternalInput")
    id_d = nc.dram_tensor("ident", [128, 128], FP16, kind="ExternalInput")
    out_d = nc.dram_tensor("out", [R, H], F32, kind="ExternalOutput")

    AX = mybir.AxisListType.X
    OP = mybir.AluOpType
    ACT = mybir.ActivationFunctionType

    def bcast(ap, reps):
        # [P, g] -> [P, g, reps] with a stride-0 innermost dim
        return bass.AP(tensor=ap.tensor, offset=ap.offset,
                       ap=[ap.ap[0], ap.ap[1], [0, reps]])

    def bcast_p1(ap, reps):
        # [P, 1] -> [P, reps] with a stride-0 free dim
        return bass.AP(tensor=ap.tensor, offset=ap.offset,
                       ap=[ap.ap[0], [0, reps]])

    with TileContext(nc) as tc, ExitStack() as ctx:
        singles = ctx.enter_context(tc.tile_pool(name="singles", bufs=1))
        xpool = ctx.enter_context(tc.tile_pool(name="xpool", bufs=4))
        spool = ctx.enter_context(tc.tile_pool(name="spool", bufs=8))
        qpool = ctx.enter_context(tc.tile_pool(name="qpool", bufs=3))
        dqpool = ctx.enter_context(tc.tile_pool(name="dqpool", bufs=3))
        tpool = ctx.enter_context(tc.tile_pool(name="tpool", bufs=3))
        hpool = ctx.enter_context(tc.tile_pool(name="hpool", bufs=3))
        opool = ctx.enter_context(tc.tile_pool(name="opool", bufs=3))
        ps_tx = ctx.enter_context(tc.tile_pool(name="ps_tx", bufs=2, space="PSUM"))
        ps_m1 = ctx.enter_context(tc.tile_pool(name="ps_m1", bufs=2, space="PSUM"))
        ps_m2 = ctx.enter_context(tc.tile_pool(name="ps_m2", bufs=1, space="PSUM"))

        state = {}
        ident = singles.tile([128, 128], FP16)
        w1_sb = singles.tile([128, E_LOC, KB1, O1], FP16)
        w2_sb = singles.tile([128, E_LOC, KB2, O2], FP16)
        rw_sb = singles.tile([128, ntiles], F32)

        # DMA issue plan (each dma_start blocks the issuing sequencer for
        # ~0.6 us, so keep the count low and off the busy engines; the ACT
        # queue is poisoned at its head by two implicit 1.3us
        # ACT_TABLE_LOADs, so nothing startup-critical goes there):
        #   SP : ident, w1[e0] kb 0-1, xq tile 0/1, kb 4-5, w2[e0], w1[e1],
        #        w2[e1], then per-tile out stores (SP is otherwise idle).
        #   DVE: rw, w1[e0] kb 2-3, kb 6-7 (issued before its first compute)
        #        so expert-0's gate_up weights arrive on two queues.
        #   GPS: remaining per-tile xq loads.
        def emit_load(it, q=None):
            x_t = xpool.tile([128, KB1, 128], FP16, tag="xq")
            (q or nc.gpsimd).dma_start(out=x_t, in_=xq_d[it])
            state[it] = {"xq": x_t}

        # Critical startup window: ONLY expert-0 gate_up weights + first xq
        # tiles. In-flight DMAs share bandwidth (not FIFO), so every byte
        # issued here delays GEMM1 of tile 0. Expert-1 weights are issued on
        # the gpsimd queue behind the xq tile-4 load, which cannot configure
        # until GEMM1 of tile 0 retires its buffer -- a natural delay.
        nc.sync.dma_start(out=ident, in_=id_d[:, :])
        nc.scalar.dma_start(out=rw_sb, in_=rw_d[:, :])
        nc.sync.dma_start(out=w1_sb[:, 0, 0], in_=w1_d[0, :, 0])
        nc.scalar.dma_start(out=w1_sb[:, 0, 1], in_=w1_d[0, :, 1])
        emit_load(0, q=nc.sync)
        nc.scalar.dma_start(out=w1_sb[:, 0, 3], in_=w1_d[0, :, 3])
        nc.sync.dma_start(out=w1_sb[:, 0, 2], in_=w1_d[0, :, 2])
        nc.scalar.dma_start(out=w1_sb[:, 0, 5], in_=w1_d[0, :, 5])
        nc.sync.dma_start(out=w1_sb[:, 0, 4], in_=w1_d[0, :, 4])
        if ntiles > 1:
            emit_load(1, q=nc.sync)
        nc.scalar.dma_start(out=w1_sb[:, 0, 7], in_=w1_d[0, :, 7])
        nc.sync.dma_start(out=w1_sb[:, 0, 6], in_=w1_d[0, :, 6])
        nc.sync.dma_start(out=w2_sb[:, 0], in_=w2_d[0])
        if E_LOC > 1:
            nc.sync.dma_start(out=w1_sb[:, 1], in_=w1_d[1])
            nc.sync.dma_start(out=w2_sb[:, 1], in_=w2_d[1])

        # PE warmup: open the HAM clock gate / ramp the p-state while the
        # first weight DMAs are in flight. Uses a memset scratch tile (not
        # the DMA'd identity) so it has no dependency on any input DMA.
        warm_in = singles.tile([128, 128], FP16)
        nc.gpsimd.memset(warm_in, 0.0)
        warm_ps = ps_tx.tile([128, KB2, 128], FP16, tag="ps_tx")
        for _ in range(WARMUP):
            nc.tensor.transpose(warm_ps[:, 0], warm_in, warm_in)

        def emit_g1(it):
            """GEMM1 for tile `it`: xqT (stationary, host-transposed) x w1."""
            e = it // tpe
            xq = state[it]["xq"]
            ps_gu = ps_m1.tile([128, 1024], F32, tag="ps_gu")
            # gate half first so silu can start at GEMM1's halfway point
            for kb in range(KB1):
                nc.tensor.matmul(ps_gu[:, 0:512], xq[:, kb], w1_sb[:, e, kb, 0:512],
                                 start=(kb == 0), stop=(kb == KB1 - 1))
            for kb in range(KB1):
                nc.tensor.matmul(ps_gu[:, 512:1024], xq[:, kb],
                                 w1_sb[:, e, kb, 512:1024],
                                 start=(kb == 0), stop=(kb == KB1 - 1))
            state[it]["gu"] = ps_gu

        def emit_silu(it):
            """ACT: silu(gate) -> SBUF (queued on ACT ahead of older copies)."""
            st = state[it]
            h_t = hpool.tile([128, I_DIM], F32, tag="h")
            nc.scalar.activation(out=h_t, in_=st["gu"][:, 0:512], func=ACT.Silu)
            st["h"] = h_t

        def emit_hchain(it):
            """DVE: h = silu*up, fp8 quant-dequant (the /2 trick), fp16 out."""
            st = state[it]
            h_t = st["h"]
            nc.vector.tensor_tensor(out=h_t, in0=h_t, in1=st["gu"][:, 512:1024],
                                    op=OP.mult)
            amax_h = spool.tile([128, KB2], F32, tag="amax_h")
            nc.vector.tensor_reduce(
                out=amax_h, in_=h_t.rearrange("p (g j) -> p g j", j=128),
                axis=AX, op=OP.max, apply_absolute_value=True)
            scl2h = spool.tile([128, KB2], F32, tag="scl_h")
            nc.vector.tensor_scalar(out=scl2h, in0=amax_h, scalar1=1e-10,
                                    scalar2=2.0 / FP8_MAX, op0=OP.max,
                                    op1=OP.mult)
            inv2h = spool.tile([128, KB2], F32, tag="inv_h")
            nc.vector.reciprocal(inv2h, scl2h)
            # fold the routing weight into the dequant scale
            # (rw * (hq @ w2) == (rw*hq) @ w2, modulo fp16 rounding of hq)
            sclrw = spool.tile([128, KB2], F32, tag="sclrw")
            nc.vector.tensor_tensor(out=sclrw, in0=scl2h,
                                    in1=bcast_p1(rw_sb[:, it:it + 1], KB2),
                                    op=OP.mult)
            # quantize + dequantize in kb-halves so the PE transposes of the
            # first half can start while DVE works on the second (tail path)
            q8h = qpool.tile([128, KB2, 128], FP8, tag="q8_h")
            hq = dqpool.tile([128, KB2, 128], FP16, tag="hq")
            hg = h_t.rearrange("p (g j) -> p g j", j=128)
            for ha in range(2):
                g = slice(ha * 2, ha * 2 + 2)
                nc.vector.tensor_tensor(out=q8h[:, g], in0=hg[:, g],
                                        in1=bcast(inv2h[:, g], 128), op=OP.mult)
                nc.vector.tensor_tensor(out=hq[:, g], in0=q8h[:, g],
                                        in1=bcast(sclrw[:, g], 128), op=OP.mult)
            st["hq"] = hq

        def emit_th(it):
            """PE transpose of hq + ACT copy to SBUF, in kb-halves."""
            st = state[it]
            psh = ps_tx.tile([128, KB2, 128], FP16, tag="ps_tx")
            hT = tpool.tile([128, KB2, 128], FP16, tag="hT")
            for ha in range(2):
                for kb in (ha * 2, ha * 2 + 1):
                    nc.tensor.transpose(psh[:, kb], st["hq"][:, kb, :], ident)
                g = slice(ha * 2, ha * 2 + 2)
                nc.scalar.copy(out=hT[:, g], in_=psh[:, g])
            st["hT"] = hT

        def emit_g2(it, tail=False):
            """GEMM2 (routing weight pre-folded into hq) + PSUM->DRAM store.

            tail=True (last two tiles): use the ps_m1 pool, which is free
            once GEMM1s are done -- ps_m2 (bufs=1) would stall on the
            previous store's drain with no GEMM1 left to cover it."""
            e = it // tpe
            r0 = it * 128
            st = state.pop(it)
            hT = st["hT"]
            pool = ps_m1 if tail else ps_m2
            ps_o = pool.tile([128, 1024], F32, tag="ps_gu" if tail else "ps_o")
            o_t = opool.tile([128, H], F32, tag="o")
            if tail:
                for half in range(2):
                    sl = slice(half * 512, half * 512 + 512)
                    for kb in range(KB2):
                        nc.tensor.matmul(ps_o[:, sl], hT[:, kb],
                                         w2_sb[:, e, kb, sl],
                                         start=(kb == 0), stop=(kb == KB2 - 1))
                    nc.scalar.activation(out=o_t[:, sl], in_=ps_o[:, sl],
                                         func=ACT.Copy)
                    nc.sync.dma_start(out=out_d[r0:r0 + 128, sl], in_=o_t[:, sl])
                return
            for kb in range(KB2):
                nc.tensor.matmul(ps_o[:, 0:512], hT[:, kb], w2_sb[:, e, kb, 0:512],
                                 start=(kb == 0), stop=(kb == KB2 - 1))
                nc.tensor.matmul(ps_o[:, 512:1024], hT[:, kb],
                                 w2_sb[:, e, kb, 512:1024],
                                 start=(kb == 0), stop=(kb == KB2 - 1))
            nc.scalar.activation(out=o_t[:, 0:512], in_=ps_o[:, 0:512],
                                 func=ACT.Copy)
            nc.scalar.activation(out=o_t[:, 512:1024], in_=ps_o[:, 512:1024],
                                 func=ACT.Copy)
            nc.sync.dma_start(out=out_d[r0:r0 + 128, :], in_=o_t)

        # Software pipeline (per-engine queues execute in emission order):
        #   PE  per step: G1_i, Th_{i-1}, G2_{i-2}
        #   ACT per step: silu_i, hTcopy_{i-1}, outcopy_{i-2}
        #   DVE per step: hchain_i
        # Every cross-engine dependency has >= one full GEMM1 of slack.
        if ntiles > 2:
            emit_load(2)
        for it in range(ntiles):
            emit_g1(it)
            emit_silu(it)
            if it >= 1:
                emit_th(it - 1)
            if it >= 2:
                emit_g2(it - 2)
            emit_hchain(it)
            if it + 3 < ntiles:
                emit_load(it + 3)
        # tail: G2(n-2) first (its hT is ready), then the transpose of the
        # last tile (which must wait for its h-chain), then a split G2.
        if ntiles >= 2:
            emit_g2(ntiles - 2, tail=True)
        emit_th(ntiles - 1)
        emit_g2(ntiles - 1, tail=True)

    if do_compile:
        nc.compile()
    return nc


def _get_program(cap: int):
    if cap not in _PROGRAM_CACHE:
        _PROGRAM_CACHE[cap] = _build_program(cap)
    return _PROGRAM_CACHE[cap]


def _dequant_weight(w, s, g=G):
    E_, O_, K_ = w.shape
    wb = w.reshape(E_, O_ // g, g, K_ // g, g)
    return (wb * s[:, :, None, :, None]).reshape(E_, O_, K_)


def _qdq_act(x):
    """Exact e4m3fn per-token/per-128-group quant-dequant (matches jax CPU)."""
    Tn, K = x.shape
    xg = x.reshape(Tn, K // G, G)
    amax = np.max(np.abs(xg), axis=-1, keepdims=True)
    scale = np.maximum(amax, 1e-10) / FP8_MAX
    q = np.clip(xg / scale, -FP8_MAX, FP8_MAX).astype(ml_dtypes.float8_e4m3fn)
    return (q.astype(np.float32) * scale).reshape(Tn, K)


def _prep(inputs):
    hs = np.ascontiguousarray(np.asarray(inputs["hidden_states"], np.float32))
    idx = np.asarray(inputs["top_k_index"]).astype(np.int64)
    tkw = np.asarray(inputs["top_k_weights"], np.float32)
    gup = np.asarray(inputs["gate_up_proj"], np.float32)
    gup_s = np.asarray(inputs["gate_up_proj_scale_inv"], np.float32)
    dn = np.asarray(inputs["down_proj"], np.float32)
    dn_s = np.asarray(inputs["down_proj_scale_inv"], np.float32)

    # routing tables: merge duplicate (token, expert) pairs (the reference
    # sums top-k weights per expert), then group by expert
    flat_e = idx.reshape(-1)
    flat_t = np.repeat(np.arange(T, dtype=np.int64), TK)
    flat_w = tkw.reshape(-1).astype(np.float64)
    key = flat_e * T + flat_t
    uk, inv = np.unique(key, return_inverse=True)
    sw = np.bincount(inv, weights=flat_w).astype(np.float32)
    se = (uk // T).astype(np.int64)
    st = (uk % T).astype(np.int64)
    counts = np.bincount(se, minlength=E)
    cap = int(np.ceil(max(int(counts.max()), 1) / 128.0) * 128)
    R = E_LOC * cap
    ntiles = R // 128

    starts = np.zeros(E + 1, np.int64)
    np.cumsum(counts, out=starts[1:])

    # exact activation quant-dequant once for all tokens, then fp16
    xq_full = _qdq_act(hs).astype(np.float16)          # [T, H]

    # weights: exact f32 block dequant, then fp16 in [K, O] layout
    w1_full = _dequant_weight(gup, gup_s)              # [E, O1, H]
    w1_t = np.ascontiguousarray(
        w1_full.transpose(0, 2, 1).reshape(E, KB1, 128, O1).transpose(0, 2, 1, 3)
    ).astype(np.float16)                               # [E, 128, KB1, O1]
    w2_full = _dequant_weight(dn, dn_s)                # [E, O2, I]
    w2_t = np.ascontiguousarray(
        w2_full.transpose(0, 2, 1).reshape(E, KB2, 128, O2).transpose(0, 2, 1, 3)
    ).astype(np.float16)                               # [E, 128, KB2, O2]

    ident = np.eye(128, dtype=np.float16)

    in_maps = []
    tok_core = []      # per-core valid token ids (concatenated per expert)
    nvalid_core = []   # per-core list of (row_offset, count)
    for c in range(NCORES):
        rows_idx = np.zeros(R, np.int64)
        rw_vec = np.zeros(R, np.float32)
        segs = []
        for j in range(E_LOC):
            e = c * E_LOC + j
            n = int(counts[e])
            s0, r0 = starts[e], j * cap
            rows_idx[r0:r0 + n] = st[s0:s0 + n]
            rw_vec[r0:r0 + n] = sw[s0:s0 + n]
            segs.append((r0, n))
        xg = xq_full[rows_idx]                         # [R, H] fp16
        # per-tile transpose: [ntiles, 128k, KB1, 128t]
        xqT = np.ascontiguousarray(
            xg.reshape(ntiles, 128, KB1, 128).transpose(0, 3, 2, 1))
        in_maps.append({
            "xq": xqT,
            "rw": np.ascontiguousarray(rw_vec.reshape(ntiles, 128).T),
            "w1": np.ascontiguousarray(w1_t[c * E_LOC:(c + 1) * E_LOC]),
            "w2": np.ascontiguousarray(w2_t[c * E_LOC:(c + 1) * E_LOC]),
            "ident": ident,
        })
        tok_core.append(rows_idx)
        nvalid_core.append(segs)
    return cap, in_maps, tok_core, nvalid_core


def _combine(results, tok_core, nvalid_core):
    out = np.zeros((T, H), np.float32)
    for c in range(NCORES):
        res = results[c]["out"]
        for (r0, n) in nvalid_core[c]:
            if n:
                np.add.at(out, tok_core[c][r0:r0 + n], res[r0:r0 + n])
    return out


def kernel_with_results(inputs, trace=False):
    from concourse.bass_utils import run_bass_kernel_spmd
    cap, in_maps, tok_core, nvalid_core = _prep(inputs)
    nc = _get_program(cap)
    bres = run_bass_kernel_spmd(nc, in_maps, core_ids=list(range(NCORES)),
                                trace=trace)
    out = _combine(bres.results, tok_core, nvalid_core)
    return out, bres


def kernel(**inputs) -> np.ndarray:
    out, _ = kernel_with_results(inputs, trace=False)
    return out
